# revision 13
# baseline (speedup 1.0000x reference)
"""Trainium2 Bass kernel for nn_DateParser (bidirectional-LSTM encoder +
attention decoder).  Data-parallel over batch: 1024 batch -> 8 cores x 128.

v2: the ENTIRE model runs on device — encoder (512-step biLSTM) and the
TY=32-step attention decoder.  Only the per-core logits (128, 32, 32) come
back to the host; the final softmax (over the batch axis, which spans all
cores) runs on host.  X ships as bf16.  This cuts per-call tunnel traffic
from ~670MB (f32 X in + donated zeros + 268MB pre out) to ~140MB.

Math tricks (all weight folding done host-side in _prep_packs):
 - sigma(x) = (1 + tanh(x/2)) / 2, with doubled state C' = 2c, H' = 2h so
   every gate needs only a plain tanh (one ACT table set for the whole
   kernel: tanh/exp/relu/copy share a set).
 - encoder h is stored transposed into SBUF-resident P2 (128b, 128n, 512t)
   in bf16 via PE transposes, so the decoder's attention context
   (a per-batch weighted sum over t) runs as DVE/GPSIMD multiply+reduce.
 - exp(relu(q)) == max(exp(q), 1), so softmax needs no extra relu pass.
 - the output-head bias b3 is constant across batch and the final softmax
   is over batch => b3 cancels; it is dropped.
"""

import time as _time

import numpy as np

B, TX, TY = 1024, 512, 32
NA, NS = 64, 128
VIN, VOUT = 64, 32
NCORES = 8
BL = B // NCORES          # 128 batch per core

_CACHE = {}


def _cfg_full():
    return dict(TX=TX, BL=BL, TY=TY, TC=16, CHN=16)


# ---------------------------------------------------------------------------
# weight packing: one f32 pack + one bf16 pack per core (identical across
# cores).  Layout must match between _pack_layout (device) and _prep_packs
# (host).
# ---------------------------------------------------------------------------

def _pack_layout():
    # name -> shape, in fixed order; all f32 except the wx_* (bf16 pack)
    f32_items = [
        ("wh_f", (NA, 4 * NA)),
        ("wh_b", (NA, 4 * NA)),
        ("w1bt_f", (NA, 10)),
        ("w1bt_b", (NA, 10)),
        ("w1at", (NS, 10)),
        ("w2rep", (128, 10)),
        ("b1rep", (128, 10)),
        ("b2rep", (128, 1)),
        ("wpih", (NS, 4 * NS)),
        ("wphh", (NS, 4 * NS)),
        ("bp4", (NS, 4)),
        ("w3t", (NS, VOUT)),
        ("ident", (128, 128)),
    ]
    bf16_items = [
        ("wx_f", (VIN + 1, 4 * NA)),
        ("wx_b", (VIN + 1, 4 * NA)),
    ]
    return f32_items, bf16_items


def _prep_packs(Wih_f, Whh_f, bih_f, bhh_f, Wih_b, Whh_b, bih_b, bhh_b,
                Wih_p, Whh_p, bih_p, bhh_p, W1, b1, W2, b2, W3, b3):
    import ml_dtypes

    def enc_weights(Wih, Whh, bih, bhh):
        b = (bih + bhh).astype(np.float32)
        scale = np.concatenate([np.full(2 * NA, 0.5, np.float32),
                                np.full(NA, 1.0, np.float32),
                                np.full(NA, 0.5, np.float32)])
        Wx = (Wih * scale[:, None]).astype(np.float32)          # (256, 64)
        Wh = (Whh * (0.5 * scale)[:, None]).astype(np.float32)  # rhs is H'=2h
        bb = (b * scale).astype(np.float32)
        wx_aug = np.concatenate([Wx.T, bb[None, :]], axis=0)    # (65, 256)
        return np.ascontiguousarray(wx_aug), np.ascontiguousarray(Wh.T)

    wxf, whf = enc_weights(Wih_f, Whh_f, bih_f, bhh_f)
    wxb, whb = enc_weights(Wih_b, Whh_b, bih_b, bhh_b)

    W1 = W1.astype(np.float32)
    W1a, W1b = W1[:, :NS], W1[:, NS:]
    # P2 holds H' = 2h -> fold 0.5 into everything that consumes it
    w1bt_f = np.ascontiguousarray(0.5 * W1b[:, :NA].T)      # (64, 10)
    w1bt_b = np.ascontiguousarray(0.5 * W1b[:, NA:].T)      # (64, 10)
    w1at = np.ascontiguousarray(0.5 * W1a.T)                # (128, 10) s=H'/2
    w2rep = np.ascontiguousarray(np.tile(W2[0].astype(np.float32), (128, 1)))
    b1rep = np.ascontiguousarray(np.tile(b1.astype(np.float32), (128, 1)))
    b2rep = np.full((128, 1), float(b2[0]), np.float32)

    # decoder LSTM: gate scale s_g (0.5 for i,f,o; 1.0 for g); operands are
    # CTX' = 2*ctx and S' = 2*s, so both W get an extra 0.5
    sg = np.concatenate([np.full(2 * NS, 0.5, np.float32),
                         np.full(NS, 1.0, np.float32),
                         np.full(NS, 0.5, np.float32)])
    bp = (bih_p + bhh_p).astype(np.float32)
    wpih = np.ascontiguousarray((0.5 * sg[:, None] * Wih_p).T)  # (128, 512)
    wphh = np.ascontiguousarray((0.5 * sg[:, None] * Whh_p).T)  # (128, 512)
    bp4 = np.ascontiguousarray((sg * bp).reshape(4, NS).T)      # (128, 4)
    w3t = np.ascontiguousarray(0.5 * W3.astype(np.float32).T)   # (128, 32)
    ident = np.eye(128, dtype=np.float32)

    vals = dict(wh_f=whf, wh_b=whb, w1bt_f=w1bt_f, w1bt_b=w1bt_b,
                w1at=w1at, w2rep=w2rep, b1rep=b1rep, b2rep=b2rep,
                wpih=wpih, wphh=wphh, bp4=bp4, w3t=w3t, ident=ident,
                wx_f=wxf, wx_b=wxb)

    f32_items, bf16_items = _pack_layout()
    pkf = np.concatenate([vals[n].astype(np.float32).ravel()
                          for n, _ in f32_items])
    pkb = np.concatenate([vals[n].ravel() for n, _ in bf16_items]
                         ).astype(ml_dtypes.bfloat16)
    return pkf, pkb


# ---------------------------------------------------------------------------
# device program
# ---------------------------------------------------------------------------

def _build(cfg, num_devices=NCORES):
    import concourse.bass as bass  # noqa: F401  (registers things)
    import concourse.bacc as bacc
    import concourse.mybir as mybir
    from concourse import tile

    tx, bl, ty, tcn, chn = (cfg["TX"], cfg["BL"], cfg["TY"], cfg["TC"],
                            cfg["CHN"])
    nchunk = tx // tcn
    nc = bacc.Bacc("TRN2", target_bir_lowering=False, debug=False,
                   num_devices=num_devices)
    f32 = mybir.dt.float32
    bf16 = mybir.dt.bfloat16
    TH = mybir.ActivationFunctionType.Tanh
    EXP = mybir.ActivationFunctionType.Exp
    RELU = mybir.ActivationFunctionType.Relu
    AD, MU = mybir.AluOpType.add, mybir.AluOpType.mult
    AXX = mybir.AxisListType.X

    xt = nc.dram_tensor("xt", [tx, VIN, bl], bf16, kind="ExternalInput").ap()
    f32_items, bf16_items = _pack_layout()
    nf32 = sum(int(np.prod(s)) for _, s in f32_items)
    nbf = sum(int(np.prod(s)) for _, s in bf16_items)
    pkf = nc.dram_tensor("pkf", [nf32], f32, kind="ExternalInput").ap()
    pkb = nc.dram_tensor("pkb", [nbf], bf16, kind="ExternalInput").ap()
    lgt = nc.dram_tensor("lgt", [bl, ty, VOUT], f32,
                         kind="ExternalOutput").ap()

    with tile.TileContext(nc) as tc:
        with (
            tc.tile_pool(name="persist", bufs=1) as pp,
        ):
            # ---- load weight packs into SBUF ----
            W = {}
            off = 0
            for n, shp in f32_items:
                t = pp.tile(list(shp), f32, name=n, tag=n)
                sz = int(np.prod(shp))
                nc.sync.dma_start(
                    t[:], pkf[off:off + sz].rearrange("(p c) -> p c",
                                                      p=shp[0]))
                W[n] = t
                off += sz
            off = 0
            for n, shp in bf16_items:
                t = pp.tile(list(shp), bf16, name=n, tag=n)
                sz = int(np.prod(shp))
                nc.sync.dma_start(
                    t[:], pkb[off:off + sz].rearrange("(p c) -> p c",
                                                      p=shp[0]))
                W[n] = t
                off += sz

            # ---- persistent state ----
            P2 = pp.tile([bl, NS, tx], bf16, name="P2", tag="P2")
            PP = pp.tile([bl, tx, 10], bf16, name="PPm", tag="PPm")
            s_sb = pp.tile([NS, bl], f32, name="s", tag="s")
            c_sb = pp.tile([NS, bl], f32, name="cdec", tag="cdec")
            ctx_sb = pp.tile([NS, bl], f32, name="ctx", tag="ctx")
            Lout = pp.tile([bl, ty, VOUT], f32, name="Lout", tag="Lout")
            nc.gpsimd.memset(s_sb[:], 0.0)
            nc.gpsimd.memset(c_sb[:], 0.0)

            # ================= encoder =================
            with (
                tc.tile_pool(name="xbuf", bufs=1) as xpool,
                tc.tile_pool(name="ppht", bufs=1) as ppht,
                tc.tile_pool(name="enc_wk", bufs=2) as wk,
                tc.tile_pool(name="enc_h", bufs=4) as hp,
                tc.tile_pool(name="enc_z", bufs=3, space="PSUM") as zp,
                tc.tile_pool(name="enc_tr", bufs=3, space="PSUM") as trp,
            ):
                PPh = {}
                for d in ("f", "b"):
                    PPh[d] = ppht.tile([bl, tx, 10], bf16, name=f"PP{d}",
                                       tag=f"PP{d}")
                xbuf = {}
                for d in ("f", "b"):
                    for s in (0, 1):
                        t = xpool.tile([VIN + 1, tcn, bl], bf16,
                                       name=f"x{d}{s}", tag=f"x{d}{s}")
                        nc.gpsimd.memset(t[VIN:VIN + 1, :, :], 1.0)
                        xbuf[d, s] = t

                h0 = pp.tile([NA, bl], f32, name="h0", tag="h0")
                nc.gpsimd.memset(h0[:], 0.0)
                cstate = {}
                for d in ("f", "b"):
                    cstate[d] = pp.tile([NA, bl], f32, name=f"c{d}",
                                        tag=f"c{d}")
                    nc.gpsimd.memset(cstate[d][:], 0.0)
                hprev = {"f": h0, "b": h0}

                for ci in range(nchunk):
                    nc.sync.dma_start(
                        xbuf["f", ci % 2][0:VIN, :, :],
                        xt[tcn * ci:tcn * (ci + 1), :, :].rearrange(
                            "t v b -> v t b"))
                    nc.sync.dma_start(
                        xbuf["b", ci % 2][0:VIN, :, :],
                        xt[tx - tcn * (ci + 1):tx - tcn * ci, :, :].rearrange(
                            "t v b -> v t b"))
                    for tl in range(tcn):
                        for d in ("f", "b"):
                            if d == "f":
                                t_act = tcn * ci + tl
                                xcol = tl
                            else:
                                t_act = tx - 1 - (tcn * ci + tl)
                                xcol = tcn - 1 - tl
                            xrhs = xbuf[d, ci % 2][:, xcol, :]
                            z = zp.tile([NA, 4 * bl], f32, name="z", tag="z")
                            for g in range(4):
                                cs = slice(g * bl, (g + 1) * bl)
                                ws = slice(g * NA, (g + 1) * NA)
                                nc.tensor.matmul(z[:, cs],
                                                 W[f"wx_{d}"][:, ws], xrhs,
                                                 start=True, stop=False)
                                nc.tensor.matmul(z[:, cs],
                                                 W[f"wh_{d}"][:, ws],
                                                 hprev[d][:], start=False,
                                                 stop=True)
                            T = wk.tile([NA, 4 * bl], f32, name="T", tag="T")
                            nc.scalar.activation(T[:], z[:], TH)
                            ti = T[:, 0:bl]
                            tf = T[:, bl:2 * bl]
                            tg = T[:, 2 * bl:3 * bl]
                            to = T[:, 3 * bl:4 * bl]
                            m1 = wk.tile([NA, bl], f32, name="m1", tag="m1")
                            m2 = wk.tile([NA, bl], f32, name="m2", tag="m2")
                            nc.vector.scalar_tensor_tensor(
                                m1[:], tf, 1.0, cstate[d][:], AD, MU)
                            nc.vector.scalar_tensor_tensor(
                                m2[:], ti, 1.0, tg, AD, MU)
                            nc.vector.scalar_tensor_tensor(
                                cstate[d][:], m1[:], 0.5, m2[:], MU, AD)
                            tcell = wk.tile([NA, bl], f32, name="tc",
                                            tag="tc")
                            nc.scalar.activation(tcell[:], cstate[d][:], TH,
                                                 scale=0.5)
                            hnew = hp.tile([NA, bl], f32, name="h", tag="h")
                            nc.vector.scalar_tensor_tensor(
                                hnew[:], to, 1.0, tcell[:], AD, MU)
                            # transpose H' into P2 column t (bf16), and
                            # accumulate PP half via PE
                            f0 = 0 if d == "f" else NA
                            trt = trp.tile([bl, NA + 10], f32, name="tr",
                                           tag="tr")
                            nc.tensor.transpose(
                                trt[:, 0:NA], hnew[:],
                                W["ident"][0:NA, 0:NA])
                            nc.tensor.matmul(trt[:, NA:NA + 10], hnew[:],
                                             W[f"w1bt_{d}"][:],
                                             start=True, stop=True)
                            nc.scalar.copy(
                                P2[:, f0:f0 + NA, t_act], trt[:, 0:NA])
                            nc.scalar.copy(
                                PPh[d][:, t_act, :], trt[:, NA:NA + 10])
                            hprev[d] = hnew

                # PP = PPf + PPb + b1
                nc.vector.scalar_tensor_tensor(
                    PP[:], PPh["f"][:], 1.0, PPh["b"][:], MU, AD)
                nc.vector.scalar_tensor_tensor(
                    PP[:], PP[:], 1.0,
                    W["b1rep"][0:bl, :].unsqueeze(1).broadcast_to(
                        (bl, tx, 10)), MU, AD)

            # ================= decoder =================
            with (
                tc.tile_pool(name="dec_wk", bufs=1) as dw,
                tc.tile_pool(name="dec_tmp", bufs=2) as dtmp,
                tc.tile_pool(name="dec_z", bufs=2, space="PSUM") as dzp,
                tc.tile_pool(name="psmall", bufs=1, space="PSUM") as psmall,
            ):
                E = dw.tile([bl, tx, 10], bf16, name="E", tag="E")
                qf = dw.tile([bl, tx], f32, name="qf", tag="qf")
                wexp = dw.tile([bl, tx], f32, name="wexp", tag="wexp")
                alph = dw.tile([bl, tx], bf16, name="alph", tag="alph")
                Ssum = dw.tile([bl, 1], f32, name="Ssum", tag="Ssum")
                rS = dw.tile([bl, 1], f32, name="rS", tag="rS")
                ctxT = dw.tile([bl, NS], f32, name="ctxT", tag="ctxT")
                ps_sb = dw.tile([bl, 10], f32, name="ps_sb", tag="ps_sb")

                nch = NS // chn
                for t in range(ty):
                    # PS = s @ (0.5*W1a.T)  -> (bl, 10)
                    pst = psmall.tile([bl, 128], f32, name="pst", tag="pst")
                    nc.tensor.matmul(pst[:, 0:10], s_sb[:], W["w1at"][:],
                                     start=True, stop=True)
                    nc.scalar.copy(ps_sb[:], pst[:, 0:10])
                    # E = tanh(PP + PS)
                    nc.vector.scalar_tensor_tensor(
                        E[:], PP[:], 1.0,
                        ps_sb.unsqueeze(1).broadcast_to((bl, tx, 10)),
                        MU, AD)
                    nc.scalar.activation(E[:], E[:], TH)
                    # q = sum_j E*W2
                    nc.vector.scalar_tensor_tensor(
                        E[:], E[:], 1.0,
                        W["w2rep"][0:bl, :].unsqueeze(1).broadcast_to(
                            (bl, tx, 10)), MU, MU)
                    nc.vector.tensor_reduce(qf[:], E[:], AXX, AD)
                    # w = exp(q + b2), relu folded: max(w, 1); S = sum w
                    nc.scalar.activation(wexp[:], qf[:], EXP,
                                         bias=W["b2rep"][0:bl, 0:1])
                    nc.vector.tensor_scalar(wexp[:], wexp[:], 1.0, 0.0,
                                            mybir.AluOpType.max, AD,
                                            accum_out=Ssum[:])
                    nc.vector.reciprocal(rS[:], Ssum[:])
                    nc.vector.tensor_scalar(alph[:], wexp[:], rS[:], None,
                                            MU)
                    # CTX' = sum_t alpha * P2
                    for cci in range(nch):
                        n0 = cci * chn
                        tmp = dtmp.tile([bl, chn, tx], bf16, name="ctmp",
                                        tag="ctmp")
                        nc.vector.scalar_tensor_tensor(
                            tmp[:], P2[:, n0:n0 + chn, :], 1.0,
                            alph.unsqueeze(1).broadcast_to((bl, chn, tx)),
                            MU, MU)
                        nc.vector.tensor_reduce(
                            ctxT[:, n0:n0 + chn], tmp[:], AXX, AD)
                    # transpose ctxT -> (NS, bl)
                    ctp = psmall.tile([NS, bl], f32, name="ctp", tag="ctp")
                    nc.tensor.transpose(ctp[:], ctxT[:], W["ident"][:])
                    nc.scalar.copy(ctx_sb[:], ctp[:])
                    # gates
                    z = dzp.tile([NS, 4 * bl], f32, name="zd", tag="zd")
                    for g in range(4):
                        cs = slice(g * bl, (g + 1) * bl)
                        ws = slice(g * NS, (g + 1) * NS)
                        nc.tensor.matmul(z[:, cs], W["wpih"][:, ws],
                                         ctx_sb[:], start=True, stop=False)
                        nc.tensor.matmul(z[:, cs], W["wphh"][:, ws],
                                         s_sb[:], start=False, stop=True)
                    T = dw.tile([NS, 4 * bl], f32, name="Td", tag="Td")
                    for g in range(4):
                        cs = slice(g * bl, (g + 1) * bl)
                        nc.scalar.activation(T[:, cs], z[:, cs], TH,
                                             bias=W["bp4"][:, g:g + 1])
                    ti = T[:, 0:bl]
                    tf = T[:, bl:2 * bl]
                    tg = T[:, 2 * bl:3 * bl]
                    to = T[:, 3 * bl:4 * bl]
                    m1 = dw.tile([NS, bl], f32, name="m1d", tag="m1d")
                    m2 = dw.tile([NS, bl], f32, name="m2d", tag="m2d")
                    nc.vector.scalar_tensor_tensor(m1[:], tf, 1.0, c_sb[:],
                                                   AD, MU)
                    nc.vector.scalar_tensor_tensor(m2[:], ti, 1.0, tg, AD,
                                                   MU)
                    nc.vector.scalar_tensor_tensor(c_sb[:], m1[:], 0.5,
                                                   m2[:], MU, AD)
                    tcell = dw.tile([NS, bl], f32, name="tcd", tag="tcd")
                    nc.scalar.activation(tcell[:], c_sb[:], TH, scale=0.5)
                    nc.vector.scalar_tensor_tensor(s_sb[:], to, 1.0,
                                                   tcell[:], AD, MU)
                    # logits
                    lp = psmall.tile([bl, 128], f32, name="lp", tag="lp")
                    nc.tensor.matmul(lp[:, 0:VOUT], s_sb[:], W["w3t"][:],
                                     start=True, stop=True)
                    nc.scalar.copy(Lout[:, t, :], lp[:, 0:VOUT])

                nc.sync.dma_start(lgt[:], Lout[:])

    nc.compile()
    return nc


def _get_nc():
    if "nc" not in _CACHE:
        _CACHE["nc"] = _build(_cfg_full())
    return _CACHE["nc"]


# ---------------------------------------------------------------------------
# jit runner (cached across calls)
# ---------------------------------------------------------------------------

def _run_cached(nc, in_maps):
    import jax
    import numpy as _np
    from jax.sharding import Mesh, PartitionSpec
    from jax.experimental.shard_map import shard_map
    from concourse import bass2jax as b2j

    if "runner" not in _CACHE:
        b2j.install_neuronx_cc_hook()
        import concourse.mybir as mybir
        pname = (nc.partition_id_tensor.name
                 if nc.partition_id_tensor else None)
        in_names, out_names, out_avals = [], [], []
        for alloc in nc.m.functions[0].allocations:
            if not isinstance(alloc, mybir.MemoryLocationSet):
                continue
            name = alloc.memorylocations[0].name
            if alloc.kind == "ExternalInput":
                if name != pname:
                    in_names.append(name)
            elif alloc.kind == "ExternalOutput":
                out_names.append(name)
                out_avals.append(jax.core.ShapedArray(
                    tuple(alloc.tensor_shape), mybir.dt.np(alloc.dtype)))
        n_params = len(in_names)
        all_names = in_names + out_names
        if pname is not None:
            all_names = all_names + [pname]

        def _body(*args):
            ops = list(args)
            if pname is not None:
                ops.append(b2j.partition_id_tensor())
            outs = b2j._bass_exec_p.bind(
                *ops, out_avals=tuple(out_avals), in_names=tuple(all_names),
                out_names=tuple(out_names), lowering_input_output_aliases=(),
                sim_require_finite=True, sim_require_nnan=True, nc=nc)
            return tuple(outs)

        devices = jax.devices()[:NCORES]
        mesh = Mesh(_np.asarray(devices), ("core",))
        nio = n_params + len(out_names)
        sharded = jax.jit(
            shard_map(_body, mesh=mesh,
                      in_specs=(PartitionSpec("core"),) * nio,
                      out_specs=(PartitionSpec("core"),) * len(out_names),
                      check_rep=False),
            donate_argnums=tuple(range(n_params, nio)), keep_unused=True)
        _CACHE["runner"] = (sharded, in_names, out_names, out_avals, n_params)

    sharded, in_names, out_names, out_avals, n_params = _CACHE["runner"]
    key = id(in_maps)
    if _CACHE.get("concat_key") != key:
        _CACHE["concat_in"] = [
            _np.concatenate([_np.asarray(m[n]) for m in in_maps], axis=0)
            for n in in_names]
        _CACHE["concat_key"] = key
    concat_in = _CACHE["concat_in"]
    concat_zeros = [
        _np.zeros((NCORES * a.shape[0], *a.shape[1:]), a.dtype)
        for a in out_avals]
    out_arrs = sharded(*concat_in, *concat_zeros)
    return [
        {n: _np.asarray(out_arrs[i]).reshape(NCORES, *out_avals[i].shape)[c]
         for i, n in enumerate(out_names)}
        for c in range(NCORES)
    ]


# ---------------------------------------------------------------------------
# entry point
# ---------------------------------------------------------------------------

def kernel(X, Wih_f, Whh_f, bih_f, bhh_f, Wih_b, Whh_b, bih_b, bhh_b,
           Wih_p, Whh_p, bih_p, bhh_p, W1, b1, W2, b2, W3, b3):
    import ml_dtypes

    _t = {}; _t0 = _time.time()
    nc = _get_nc()
    _t['build'] = _time.time() - _t0; _t0 = _time.time()

    pkf, pkb = _prep_packs(Wih_f, Whh_f, bih_f, bhh_f, Wih_b, Whh_b,
                           bih_b, bhh_b, Wih_p, Whh_p, bih_p, bhh_p,
                           W1, b1, W2, b2, W3, b3)

    in_maps = []
    for c in range(NCORES):
        xc = X[c * BL:(c + 1) * BL]                      # (128, 512, 64)
        xtc = np.ascontiguousarray(
            xc.transpose(1, 2, 0)).astype(ml_dtypes.bfloat16)
        in_maps.append({"xt": xtc, "pkf": pkf, "pkb": pkb})

    _t['prep'] = _time.time() - _t0; _t0 = _time.time()
    results = _run_cached(nc, in_maps)
    _t['spmd'] = _time.time() - _t0; _t0 = _time.time()
    _CACHE["last_results"] = results
    _CACHE["last_in_maps"] = in_maps

    # logits (B, TY, VOUT); b2-style constant shifts cancel in the
    # batch-axis softmax (b3 already dropped on device)
    L = np.empty((B, TY, VOUT), np.float32)
    for c in range(NCORES):
        L[c * BL:(c + 1) * BL] = results[c]["lgt"]
    Lm = L.max(axis=0, keepdims=True)
    em = np.exp(L - Lm)
    out = em / em.sum(axis=0, keepdims=True)
    _t['decoder'] = _time.time() - _t0
    _CACHE['timers'] = _t
    return np.ascontiguousarray(out)


# revision 18
# speedup vs baseline: 1.6671x; 1.6671x over previous
"""Trainium2 Bass kernel for nn_DateParser (bidirectional-LSTM encoder +
attention decoder).  Data-parallel over batch: 1024 batch -> 8 cores x 128.

v2: the ENTIRE model runs on device — encoder (512-step biLSTM) and the
TY=32-step attention decoder.  Only the per-core logits (128, 32, 32) come
back to the host; the final softmax (over the batch axis, which spans all
cores) runs on host.  X ships as bf16.  This cuts per-call tunnel traffic
from ~670MB (f32 X in + donated zeros + 268MB pre out) to ~140MB.

Math tricks (all weight folding done host-side in _prep_packs):
 - sigma(x) = (1 + tanh(x/2)) / 2, with doubled state C' = 2c, H' = 2h so
   every gate needs only a plain tanh (one ACT table set for the whole
   kernel: tanh/exp/relu/copy share a set).
 - encoder h is stored transposed into SBUF-resident P2 (128b, 128n, 512t)
   in bf16 via PE transposes, so the decoder's attention context
   (a per-batch weighted sum over t) runs as DVE/GPSIMD multiply+reduce.
 - exp(relu(q)) == max(exp(q), 1), so softmax needs no extra relu pass.
 - the output-head bias b3 is constant across batch and the final softmax
   is over batch => b3 cancels; it is dropped.
"""

import time as _time

import numpy as np

B, TX, TY = 1024, 512, 32
NA, NS = 64, 128
VIN, VOUT = 64, 32
NCORES = 8
BL = B // NCORES          # 128 batch per core

_CACHE = {}


def _cfg_full():
    return dict(TX=TX, BL=BL, TY=TY, TC=16, CHN=16)


# ---------------------------------------------------------------------------
# weight packing: one f32 pack + one bf16 pack per core (identical across
# cores).  Layout must match between _pack_layout (device) and _prep_packs
# (host).
# ---------------------------------------------------------------------------

def _pack_layout():
    # name -> shape, in fixed order; all f32 except the wx_* (bf16 pack)
    f32_items = [
        ("wh_f", (NA, 4 * NA)),
        ("wh_b", (NA, 4 * NA)),
        ("w1bt_f", (NA, 10)),
        ("w1bt_b", (NA, 10)),
        ("w1at", (NS, 10)),
        ("w2rep", (128, 10)),
        ("b1rep", (128, 10)),
        ("b2rep", (128, 1)),
        ("wpih", (NS, 4 * NS)),
        ("wphh", (NS, 4 * NS)),
        ("bp4", (NS, 4)),
        ("w3t", (NS, VOUT)),
        ("ident", (128, 128)),
    ]
    bf16_items = [
        ("wx_f", (VIN + 1, 4 * NA)),
        ("wx_b", (VIN + 1, 4 * NA)),
    ]
    return f32_items, bf16_items


def _prep_packs(Wih_f, Whh_f, bih_f, bhh_f, Wih_b, Whh_b, bih_b, bhh_b,
                Wih_p, Whh_p, bih_p, bhh_p, W1, b1, W2, b2, W3, b3):
    import ml_dtypes

    def enc_weights(Wih, Whh, bih, bhh):
        b = (bih + bhh).astype(np.float32)
        scale = np.concatenate([np.full(2 * NA, 0.5, np.float32),
                                np.full(NA, 1.0, np.float32),
                                np.full(NA, 0.5, np.float32)])
        Wx = (Wih * scale[:, None]).astype(np.float32)          # (256, 64)
        Wh = (Whh * (0.5 * scale)[:, None]).astype(np.float32)  # rhs is H'=2h
        bb = (b * scale).astype(np.float32)
        wx_aug = np.concatenate([Wx.T, bb[None, :]], axis=0)    # (65, 256)
        return np.ascontiguousarray(wx_aug), np.ascontiguousarray(Wh.T)

    wxf, whf = enc_weights(Wih_f, Whh_f, bih_f, bhh_f)
    wxb, whb = enc_weights(Wih_b, Whh_b, bih_b, bhh_b)

    W1 = W1.astype(np.float32)
    W1a, W1b = W1[:, :NS], W1[:, NS:]
    # P2 holds H' = 2h -> fold 0.5 into everything that consumes it
    w1bt_f = np.ascontiguousarray(0.5 * W1b[:, :NA].T)      # (64, 10)
    w1bt_b = np.ascontiguousarray(0.5 * W1b[:, NA:].T)      # (64, 10)
    w1at = np.ascontiguousarray(0.5 * W1a.T)                # (128, 10) s=H'/2
    w2rep = np.ascontiguousarray(np.tile(W2[0].astype(np.float32), (128, 1)))
    b1rep = np.ascontiguousarray(np.tile(b1.astype(np.float32), (128, 1)))
    b2rep = np.full((128, 1), float(b2[0]), np.float32)

    # decoder LSTM: gate scale s_g (0.5 for i,f,o; 1.0 for g); operands are
    # CTX' = 2*ctx and S' = 2*s, so both W get an extra 0.5
    sg = np.concatenate([np.full(2 * NS, 0.5, np.float32),
                         np.full(NS, 1.0, np.float32),
                         np.full(NS, 0.5, np.float32)])
    bp = (bih_p + bhh_p).astype(np.float32)
    wpih = np.ascontiguousarray((0.5 * sg[:, None] * Wih_p).T)  # (128, 512)
    wphh = np.ascontiguousarray((0.5 * sg[:, None] * Whh_p).T)  # (128, 512)
    bp4 = np.ascontiguousarray((sg * bp).reshape(4, NS).T)      # (128, 4)
    w3t = np.ascontiguousarray(0.5 * W3.astype(np.float32).T)   # (128, 32)
    ident = np.eye(128, dtype=np.float32)

    vals = dict(wh_f=whf, wh_b=whb, w1bt_f=w1bt_f, w1bt_b=w1bt_b,
                w1at=w1at, w2rep=w2rep, b1rep=b1rep, b2rep=b2rep,
                wpih=wpih, wphh=wphh, bp4=bp4, w3t=w3t, ident=ident,
                wx_f=wxf, wx_b=wxb)

    f32_items, bf16_items = _pack_layout()
    pkf = np.concatenate([vals[n].astype(np.float32).ravel()
                          for n, _ in f32_items])
    pkb = np.concatenate([vals[n].ravel() for n, _ in bf16_items]
                         ).astype(ml_dtypes.bfloat16)
    return pkf, pkb


# ---------------------------------------------------------------------------
# device program
# ---------------------------------------------------------------------------

def _build(cfg, num_devices=NCORES):
    import concourse.bass as bass  # noqa: F401  (registers things)
    import concourse.bacc as bacc
    import concourse.mybir as mybir
    from concourse import tile

    tx, bl, ty, tcn, chn = (cfg["TX"], cfg["BL"], cfg["TY"], cfg["TC"],
                            cfg["CHN"])
    nchunk = tx // tcn
    nc = bacc.Bacc("TRN2", target_bir_lowering=False, debug=False,
                   num_devices=num_devices)
    f32 = mybir.dt.float32
    bf16 = mybir.dt.bfloat16
    fp8 = mybir.dt.float8e3
    TH = mybir.ActivationFunctionType.Tanh
    EXP = mybir.ActivationFunctionType.Exp
    RELU = mybir.ActivationFunctionType.Relu
    AD, MU = mybir.AluOpType.add, mybir.AluOpType.mult
    AXX = mybir.AxisListType.X

    xt = nc.dram_tensor("xt", [tx, VIN, bl], fp8, kind="ExternalInput").ap()
    f32_items, bf16_items = _pack_layout()
    nf32 = sum(int(np.prod(s)) for _, s in f32_items)
    nbf = sum(int(np.prod(s)) for _, s in bf16_items)
    pkf = nc.dram_tensor("pkf", [nf32], f32, kind="ExternalInput").ap()
    pkb = nc.dram_tensor("pkb", [nbf], bf16, kind="ExternalInput").ap()
    lgt = nc.dram_tensor("lgt", [bl, ty, VOUT], f32,
                         kind="ExternalOutput").ap()

    with tile.TileContext(nc) as tc:
        with (
            tc.tile_pool(name="persist", bufs=1) as pp,
        ):
            # ---- load weight packs into SBUF ----
            W = {}
            off = 0
            for n, shp in f32_items:
                t = pp.tile(list(shp), f32, name=n, tag=n)
                sz = int(np.prod(shp))
                nc.sync.dma_start(
                    t[:], pkf[off:off + sz].rearrange("(p c) -> p c",
                                                      p=shp[0]))
                W[n] = t
                off += sz
            off = 0
            for n, shp in bf16_items:
                t = pp.tile(list(shp), bf16, name=n, tag=n)
                sz = int(np.prod(shp))
                nc.sync.dma_start(
                    t[:], pkb[off:off + sz].rearrange("(p c) -> p c",
                                                      p=shp[0]))
                W[n] = t
                off += sz

            # ---- persistent state ----
            P2 = pp.tile([bl, NS, tx], bf16, name="P2", tag="P2")
            PP = pp.tile([bl, tx, 10], bf16, name="PPm", tag="PPm")
            s_sb = pp.tile([NS, bl], f32, name="s", tag="s")
            c_sb = pp.tile([NS, bl], f32, name="cdec", tag="cdec")
            ctx_sb = pp.tile([NS, bl], f32, name="ctx", tag="ctx")
            Lout = pp.tile([bl, ty, VOUT], f32, name="Lout", tag="Lout")
            nc.gpsimd.memset(s_sb[:], 0.0)
            nc.gpsimd.memset(c_sb[:], 0.0)

            # ================= encoder =================
            with (
                tc.tile_pool(name="xbuf", bufs=1) as xpool,
                tc.tile_pool(name="ppht", bufs=1) as ppht,
                tc.tile_pool(name="enc_wk", bufs=2) as wk,
                tc.tile_pool(name="enc_h", bufs=4) as hp,
                tc.tile_pool(name="enc_z", bufs=3, space="PSUM") as zp,
                tc.tile_pool(name="enc_tr", bufs=3, space="PSUM") as trp,
            ):
                PPh = {}
                for d in ("f", "b"):
                    PPh[d] = ppht.tile([bl, tx, 10], bf16, name=f"PP{d}",
                                       tag=f"PP{d}")
                xbuf = {}
                for d in ("f", "b"):
                    for s in (0, 1):
                        t = xpool.tile([VIN + 1, tcn, bl], fp8,
                                       name=f"x{d}{s}", tag=f"x{d}{s}")
                        nc.gpsimd.memset(t[VIN:VIN + 1, :, :], 1.0)
                        xbuf[d, s] = t

                h0 = pp.tile([NA, bl], f32, name="h0", tag="h0")
                nc.gpsimd.memset(h0[:], 0.0)
                cstate = {}
                for d in ("f", "b"):
                    cstate[d] = pp.tile([NA, bl], f32, name=f"c{d}",
                                        tag=f"c{d}")
                    nc.gpsimd.memset(cstate[d][:], 0.0)
                hprev = {"f": h0, "b": h0}

                for ci in range(nchunk):
                    nc.sync.dma_start(
                        xbuf["f", ci % 2][0:VIN, :, :],
                        xt[tcn * ci:tcn * (ci + 1), :, :].rearrange(
                            "t v b -> v t b"))
                    nc.sync.dma_start(
                        xbuf["b", ci % 2][0:VIN, :, :],
                        xt[tx - tcn * (ci + 1):tx - tcn * ci, :, :].rearrange(
                            "t v b -> v t b"))
                    for tl in range(tcn):
                        for d in ("f", "b"):
                            if d == "f":
                                t_act = tcn * ci + tl
                                xcol = tl
                            else:
                                t_act = tx - 1 - (tcn * ci + tl)
                                xcol = tcn - 1 - tl
                            xrhs = xbuf[d, ci % 2][:, xcol, :]
                            z = zp.tile([NA, 4 * bl], f32, name="z", tag="z")
                            for g in range(4):
                                cs = slice(g * bl, (g + 1) * bl)
                                ws = slice(g * NA, (g + 1) * NA)
                                nc.tensor.matmul(z[:, cs],
                                                 W[f"wx_{d}"][:, ws], xrhs,
                                                 start=True, stop=False)
                                nc.tensor.matmul(z[:, cs],
                                                 W[f"wh_{d}"][:, ws],
                                                 hprev[d][:], start=False,
                                                 stop=True)
                            T = wk.tile([NA, 4 * bl], f32, name="T", tag="T")
                            nc.scalar.activation(T[:], z[:], TH)
                            ti = T[:, 0:bl]
                            tf = T[:, bl:2 * bl]
                            tg = T[:, 2 * bl:3 * bl]
                            to = T[:, 3 * bl:4 * bl]
                            m1 = wk.tile([NA, bl], f32, name="m1", tag="m1")
                            m2 = wk.tile([NA, bl], f32, name="m2", tag="m2")
                            nc.vector.scalar_tensor_tensor(
                                m1[:], tf, 1.0, cstate[d][:], AD, MU)
                            nc.vector.scalar_tensor_tensor(
                                m2[:], ti, 1.0, tg, AD, MU)
                            nc.vector.scalar_tensor_tensor(
                                cstate[d][:], m1[:], 0.5, m2[:], MU, AD)
                            tcell = wk.tile([NA, bl], f32, name="tc",
                                            tag="tc")
                            nc.scalar.activation(tcell[:], cstate[d][:], TH,
                                                 scale=0.5)
                            hnew = hp.tile([NA, bl], f32, name="h", tag="h")
                            nc.vector.scalar_tensor_tensor(
                                hnew[:], to, 1.0, tcell[:], AD, MU)
                            # transpose H' into P2 column t (bf16), and
                            # accumulate PP half via PE
                            f0 = 0 if d == "f" else NA
                            trt = trp.tile([bl, NA + 10], f32, name="tr",
                                           tag="tr")
                            nc.tensor.transpose(
                                trt[:, 0:NA], hnew[:],
                                W["ident"][0:NA, 0:NA])
                            nc.tensor.matmul(trt[:, NA:NA + 10], hnew[:],
                                             W[f"w1bt_{d}"][:],
                                             start=True, stop=True)
                            nc.scalar.copy(
                                P2[:, f0:f0 + NA, t_act], trt[:, 0:NA])
                            nc.scalar.copy(
                                PPh[d][:, t_act, :], trt[:, NA:NA + 10])
                            hprev[d] = hnew

                # PP = PPf + PPb + b1
                nc.vector.scalar_tensor_tensor(
                    PP[:], PPh["f"][:], 1.0, PPh["b"][:], MU, AD)
                nc.vector.scalar_tensor_tensor(
                    PP[:], PP[:], 1.0,
                    W["b1rep"][0:bl, :].unsqueeze(1).broadcast_to(
                        (bl, tx, 10)), MU, AD)

            # ================= decoder =================
            with (
                tc.tile_pool(name="dec_wk", bufs=1) as dw,
                tc.tile_pool(name="dec_tmp", bufs=2) as dtmp,
                tc.tile_pool(name="dec_z", bufs=2, space="PSUM") as dzp,
                tc.tile_pool(name="psmall", bufs=1, space="PSUM") as psmall,
            ):
                E = dw.tile([bl, tx, 10], bf16, name="E", tag="E")
                qf = dw.tile([bl, tx], f32, name="qf", tag="qf")
                wexp = dw.tile([bl, tx], f32, name="wexp", tag="wexp")
                alph = dw.tile([bl, tx], bf16, name="alph", tag="alph")
                Ssum = dw.tile([bl, 1], f32, name="Ssum", tag="Ssum")
                rS = dw.tile([bl, 1], f32, name="rS", tag="rS")
                ctxT = dw.tile([bl, NS], f32, name="ctxT", tag="ctxT")
                ps_sb = dw.tile([bl, 10], f32, name="ps_sb", tag="ps_sb")

                nch = NS // chn
                for t in range(ty):
                    # PS = s @ (0.5*W1a.T)  -> (bl, 10)
                    pst = psmall.tile([bl, 128], f32, name="pst", tag="pst")
                    nc.tensor.matmul(pst[:, 0:10], s_sb[:], W["w1at"][:],
                                     start=True, stop=True)
                    nc.scalar.copy(ps_sb[:], pst[:, 0:10])
                    # E = tanh(PP + PS)
                    nc.vector.scalar_tensor_tensor(
                        E[:], PP[:], 1.0,
                        ps_sb.unsqueeze(1).broadcast_to((bl, tx, 10)),
                        MU, AD)
                    nc.scalar.activation(E[:], E[:], TH)
                    # q = sum_j E*W2
                    nc.vector.scalar_tensor_tensor(
                        E[:], E[:], 1.0,
                        W["w2rep"][0:bl, :].unsqueeze(1).broadcast_to(
                            (bl, tx, 10)), MU, MU)
                    nc.vector.tensor_reduce(qf[:], E[:], AXX, AD)
                    # w = exp(q + b2), relu folded: max(w, 1); S = sum w
                    nc.scalar.activation(wexp[:], qf[:], EXP,
                                         bias=W["b2rep"][0:bl, 0:1])
                    nc.vector.tensor_scalar(wexp[:], wexp[:], 1.0, 0.0,
                                            mybir.AluOpType.max, AD,
                                            accum_out=Ssum[:])
                    nc.vector.reciprocal(rS[:], Ssum[:])
                    nc.vector.tensor_scalar(alph[:], wexp[:], rS[:], None,
                                            MU)
                    # CTX' = sum_t alpha * P2
                    for cci in range(nch):
                        n0 = cci * chn
                        tmp = dtmp.tile([bl, chn, tx], bf16, name="ctmp",
                                        tag="ctmp")
                        nc.vector.scalar_tensor_tensor(
                            tmp[:], P2[:, n0:n0 + chn, :], 1.0,
                            alph.unsqueeze(1).broadcast_to((bl, chn, tx)),
                            MU, MU)
                        nc.vector.tensor_reduce(
                            ctxT[:, n0:n0 + chn], tmp[:], AXX, AD)
                    # transpose ctxT -> (NS, bl)
                    ctp = psmall.tile([NS, bl], f32, name="ctp", tag="ctp")
                    nc.tensor.transpose(ctp[:], ctxT[:], W["ident"][:])
                    nc.scalar.copy(ctx_sb[:], ctp[:])
                    # gates
                    z = dzp.tile([NS, 4 * bl], f32, name="zd", tag="zd")
                    for g in range(4):
                        cs = slice(g * bl, (g + 1) * bl)
                        ws = slice(g * NS, (g + 1) * NS)
                        nc.tensor.matmul(z[:, cs], W["wpih"][:, ws],
                                         ctx_sb[:], start=True, stop=False)
                        nc.tensor.matmul(z[:, cs], W["wphh"][:, ws],
                                         s_sb[:], start=False, stop=True)
                    T = dw.tile([NS, 4 * bl], f32, name="Td", tag="Td")
                    for g in range(4):
                        cs = slice(g * bl, (g + 1) * bl)
                        nc.scalar.activation(T[:, cs], z[:, cs], TH,
                                             bias=W["bp4"][:, g:g + 1])
                    ti = T[:, 0:bl]
                    tf = T[:, bl:2 * bl]
                    tg = T[:, 2 * bl:3 * bl]
                    to = T[:, 3 * bl:4 * bl]
                    m1 = dw.tile([NS, bl], f32, name="m1d", tag="m1d")
                    m2 = dw.tile([NS, bl], f32, name="m2d", tag="m2d")
                    nc.vector.scalar_tensor_tensor(m1[:], tf, 1.0, c_sb[:],
                                                   AD, MU)
                    nc.vector.scalar_tensor_tensor(m2[:], ti, 1.0, tg, AD,
                                                   MU)
                    nc.vector.scalar_tensor_tensor(c_sb[:], m1[:], 0.5,
                                                   m2[:], MU, AD)
                    tcell = dw.tile([NS, bl], f32, name="tcd", tag="tcd")
                    nc.scalar.activation(tcell[:], c_sb[:], TH, scale=0.5)
                    nc.vector.scalar_tensor_tensor(s_sb[:], to, 1.0,
                                                   tcell[:], AD, MU)
                    # logits
                    lp = psmall.tile([bl, 128], f32, name="lp", tag="lp")
                    nc.tensor.matmul(lp[:, 0:VOUT], s_sb[:], W["w3t"][:],
                                     start=True, stop=True)
                    nc.scalar.copy(Lout[:, t, :], lp[:, 0:VOUT])

                nc.sync.dma_start(lgt[:], Lout[:])

    nc.compile()
    return nc


def _get_nc():
    if "nc" not in _CACHE:
        _CACHE["nc"] = _build(_cfg_full())
    return _CACHE["nc"]


# ---------------------------------------------------------------------------
# jit runner (cached across calls)
# ---------------------------------------------------------------------------

def _run_cached(nc, in_maps):
    import jax
    import numpy as _np
    from jax.sharding import Mesh, PartitionSpec
    from jax.experimental.shard_map import shard_map
    from concourse import bass2jax as b2j

    if "runner" not in _CACHE:
        b2j.install_neuronx_cc_hook()
        import concourse.mybir as mybir
        pname = (nc.partition_id_tensor.name
                 if nc.partition_id_tensor else None)
        in_names, out_names, out_avals = [], [], []
        for alloc in nc.m.functions[0].allocations:
            if not isinstance(alloc, mybir.MemoryLocationSet):
                continue
            name = alloc.memorylocations[0].name
            if alloc.kind == "ExternalInput":
                if name != pname:
                    in_names.append(name)
            elif alloc.kind == "ExternalOutput":
                out_names.append(name)
                out_avals.append(jax.core.ShapedArray(
                    tuple(alloc.tensor_shape), mybir.dt.np(alloc.dtype)))
        n_params = len(in_names)
        all_names = in_names + out_names
        if pname is not None:
            all_names = all_names + [pname]

        def _body(*args):
            ops = list(args)
            if pname is not None:
                ops.append(b2j.partition_id_tensor())
            outs = b2j._bass_exec_p.bind(
                *ops, out_avals=tuple(out_avals), in_names=tuple(all_names),
                out_names=tuple(out_names), lowering_input_output_aliases=(),
                sim_require_finite=True, sim_require_nnan=True, nc=nc)
            return tuple(outs)

        devices = jax.devices()[:NCORES]
        mesh = Mesh(_np.asarray(devices), ("core",))
        nio = n_params + len(out_names)
        sharded = jax.jit(
            shard_map(_body, mesh=mesh,
                      in_specs=(PartitionSpec("core"),) * nio,
                      out_specs=(PartitionSpec("core"),) * len(out_names),
                      check_rep=False),
            donate_argnums=tuple(range(n_params, nio)), keep_unused=True)
        _CACHE["runner"] = (sharded, in_names, out_names, out_avals, n_params)

    sharded, in_names, out_names, out_avals, n_params = _CACHE["runner"]
    key = id(in_maps)
    if _CACHE.get("concat_key") != key:
        from jax.sharding import Mesh, PartitionSpec, NamedSharding
        mesh = Mesh(_np.asarray(jax.devices()[:NCORES]), ("core",))
        shd = NamedSharding(mesh, PartitionSpec("core"))
        concat = []
        for n in in_names:
            arr = _np.concatenate([_np.asarray(m[n]) for m in in_maps],
                                  axis=0)
            if n != "xt":
                # weights are tiny and constant: pre-stage on device so the
                # per-call transfer is just the activations
                arr = jax.device_put(arr, shd)
                arr.block_until_ready()
            concat.append(arr)
        _CACHE["concat_in"] = concat
        _CACHE["concat_key"] = key
    concat_in = _CACHE["concat_in"]
    concat_zeros = [
        _np.zeros((NCORES * a.shape[0], *a.shape[1:]), a.dtype)
        for a in out_avals]
    out_arrs = sharded(*concat_in, *concat_zeros)
    return [
        {n: _np.asarray(out_arrs[i]).reshape(NCORES, *out_avals[i].shape)[c]
         for i, n in enumerate(out_names)}
        for c in range(NCORES)
    ]


# ---------------------------------------------------------------------------
# entry point
# ---------------------------------------------------------------------------

def kernel(X, Wih_f, Whh_f, bih_f, bhh_f, Wih_b, Whh_b, bih_b, bhh_b,
           Wih_p, Whh_p, bih_p, bhh_p, W1, b1, W2, b2, W3, b3):
    import ml_dtypes

    _t = {}; _t0 = _time.time()
    nc = _get_nc()
    _t['build'] = _time.time() - _t0; _t0 = _time.time()

    pkf, pkb = _prep_packs(Wih_f, Whh_f, bih_f, bhh_f, Wih_b, Whh_b,
                           bih_b, bhh_b, Wih_p, Whh_p, bih_p, bhh_p,
                           W1, b1, W2, b2, W3, b3)

    in_maps = []
    for c in range(NCORES):
        xc = X[c * BL:(c + 1) * BL]                      # (128, 512, 64)
        xtc = np.ascontiguousarray(
            xc.transpose(1, 2, 0)).astype(ml_dtypes.float8_e3m4)
        in_maps.append({"xt": xtc, "pkf": pkf, "pkb": pkb})

    _t['prep'] = _time.time() - _t0; _t0 = _time.time()
    results = _run_cached(nc, in_maps)
    _t['spmd'] = _time.time() - _t0; _t0 = _time.time()
    _CACHE["last_results"] = results
    _CACHE["last_in_maps"] = in_maps

    # logits (B, TY, VOUT); b2-style constant shifts cancel in the
    # batch-axis softmax (b3 already dropped on device)
    L = np.empty((B, TY, VOUT), np.float32)
    for c in range(NCORES):
        L[c * BL:(c + 1) * BL] = results[c]["lgt"]
    Lm = L.max(axis=0, keepdims=True)
    em = np.exp(L - Lm)
    out = em / em.sum(axis=0, keepdims=True)
    _t['decoder'] = _time.time() - _t0
    _CACHE['timers'] = _t
    return np.ascontiguousarray(out)
